# revision 8
# baseline (speedup 1.0000x reference)
# Trainium2 Bass kernel for nn_Attention_80779744903426
#
# Reference computation (b=4, n=2048, c=1024, h=16, d=64):
#   qkv = x @ w_qkv ; split to q,k,v per head
#   attn = softmax(q k^T / sqrt(c)) ; out = (attn v) concat ; y = out @ w_proj + b_proj
#
# Sharding (8 cores): data-parallel over batch (4) x tensor-parallel over
# head-groups (2 groups of 8 heads, Megatron-style). Each core computes a
# partial y for its batch from its 8 heads; host sums the two partials per
# batch and adds b_proj.
#
# Per-core program (all matmuls bf16, fp32 PSUM accumulation):
#   A) Q^T/K^T = wqk^T @ x^T and V = x @ wv, kept resident in SBUF.
#   B) attention per head pair (A, B) and q-chunk of 512:
#      - S^T[k,q] built per k-tile as a 4x (64,64)-tiled PE quad
#        (heads x k-halves at tile positions {0,64}x{0,64}) writing one
#        shared 2-bank PSUM region -> all four matmuls run concurrently.
#      - exp: split between ACT (table exp, softmax scale folded) and a
#        custom 8-stage DVE op (exp(x) ~ (1+u(1+u(1/2+u/6)))^2, u=x/64)
#        so neither activation engine is the bottleneck.
#      - PV: col-tiled pair (V_A -> out partitions 0:64, V_B -> 64:128)
#        accumulating over 16 k-tiles into one PSUM bank; softmax
#        denominators via M=1 ones-matmul col-pairs into partitions 0/32
#        of a dens bank.
#      - norm: reciprocal_approx_fast on dens (direct from PSUM),
#        partition-broadcast via DRAM-bounce DMAs on the gpsimd queue,
#        one fused tensor_mul (PSUM O' x bc -> ot_sb bf16).
#   C) y = O^T @ wp interleaved into later iterations; PSUM->SBUF staging
#      copies ride the scalar engine (closer to PSUM) to keep DVE free.

import numpy as np

DIM = 1024
N = 2048
B = 4
NH = 16
HD = 64
SCALE = 1.0 / DIM**0.5

HPC = 8            # heads per core
PAIRS = HPC // 2   # head pairs
CT = 8             # contraction tiles over c=1024
ACH = 512          # phase-A n-chunk
QCH = 512          # phase-B q-chunk
NQC = N // QCH     # 4 q-chunks
KT = 16            # k tiles of 128 in attention

# k-tiles whose exp runs on the DVE custom op (rest on ACT)
S_DVE_KS = frozenset((2, 5, 8, 11, 14))

_CACHE = {}


def _register_exp_op():
    """Register the custom DVE op exp(x) ~ (poly3(x*C0))^2 at runtime."""
    import re

    from concourse import dve_ops
    from concourse.dve_spec import C0, C1, C2, One, Spec, Src0, sq

    for o in dve_ops.OPS:
        if o.name == "EXP_POLY_SQ":
            return o

    u = Src0 * C0
    h1 = u * C1 + C2
    h2 = u * h1 + One
    h3 = u * h2 + One

    def _ref(in0, in1, s0, s1, imm2):
        uu = in0 * s0
        p = 1.0 + uu * (1.0 + uu * (imm2 + uu * s1))
        return (p * p).astype(np.float32)

    spec = Spec(body=sq(h3), reference=_ref)
    probe = dve_ops.DveOp("EXP_POLY_SQ", spec, subdim=False, uops_sha={})
    dve_ops.OPS.append(probe)
    dve_ops._SUB_OPCODE_FOR_NAME["EXP_POLY_SQ"] = (
        dve_ops._CUSTOM_DVE_ROW_BASE + len(dve_ops.OPS) - 1)
    shas = {}
    for ver in ("v3", "v4"):
        try:
            probe.compile(ver)
        except ValueError as e:
            m = re.search(r"v[34]: ([0-9a-f]+) ", str(e))
            assert m, f"cannot parse sha from: {e}"
            shas[ver] = m.group(1)
    op = dve_ops.DveOp("EXP_POLY_SQ", spec, subdim=False, uops_sha=shas)
    dve_ops.OPS[-1] = op
    dve_ops.CUSTOM_DVE_SPECS["EXP_POLY_SQ"] = spec
    return op


def _build_nc():
    import concourse.bass as bass
    from concourse import bacc, mybir, tile

    f32 = mybir.dt.float32
    bf16 = mybir.dt.bfloat16
    EXP = mybir.ActivationFunctionType.Exp
    EXP_POLY = _register_exp_op()

    nc = bacc.Bacc("TRN2", target_bir_lowering=False, debug=False)

    xT_d = nc.dram_tensor("xT", [DIM, N], bf16, kind="ExternalInput").ap()
    wqk_d = nc.dram_tensor("wqk", [DIM, 1024], bf16, kind="ExternalInput").ap()
    wv_d = nc.dram_tensor("wv", [DIM, 512], bf16, kind="ExternalInput").ap()
    wp_d = nc.dram_tensor("wp", [512, DIM], bf16, kind="ExternalInput").ap()
    y_d = nc.dram_tensor("y", [N, DIM], f32, kind="ExternalOutput").ap()

    with tile.TileContext(nc) as tc:
        with (
            tc.tile_pool(name="pt", bufs=2) as ptp,       # 32KB P~ tiles
            tc.tile_pool(name="xt", bufs=3) as xtp,       # 8KB x^T chunks
            tc.tile_pool(name="wqk", bufs=1) as wqkp,
            tc.tile_pool(name="wv", bufs=1) as wvp,
            tc.tile_pool(name="wp", bufs=1) as wpp,
            tc.tile_pool(name="v", bufs=1) as vp,
            tc.tile_pool(name="ot", bufs=1) as otp,
            tc.tile_pool(name="misc", bufs=2) as miscp,
            tc.tile_pool(name="ps", bufs=1, space="PSUM") as psp,
            tc.tile_pool(name="dram", bufs=1, space="DRAM") as dp,
        ):
            xT_r = xT_d.rearrange("(t p) n -> p t n", p=128)

            # ---- static tiles; K-half of wqk + first x chunk first so the
            # K^T chains can start ASAP ----
            wqk_sb = wqkp.tile([128, CT, 1024], bf16)
            xt0 = xtp.tile([128, CT, ACH], bf16, tag="xt", name="xt")
            nc.sync.dma_start(xt0, xT_r[:, :, 0:ACH])
            for ct in range(CT):
                nc.sync.dma_start(
                    wqk_sb[:, ct, 512:1024],
                    wqk_d[128 * ct : 128 * (ct + 1), 512:1024])
            for ct in range(CT):
                nc.sync.dma_start(
                    wqk_sb[:, ct, 0:512],
                    wqk_d[128 * ct : 128 * (ct + 1), 0:512])
            wv_sb = wvp.tile([128, CT, 512], bf16)
            for ct in range(CT):
                nc.sync.dma_start(wv_sb[:, ct, :], wv_d[128 * ct : 128 * (ct + 1), :])
            wp_sb = wpp.tile([128, 4, 1024], bf16)

            v_sb = vp.tile([128, KT, HPC, HD], bf16)  # [k-part, k-tile, head, d]
            ones_sb = miscp.tile([128, 1], bf16, tag="ones", bufs=1)
            nc.vector.memset(ones_sb, 1.0)

            ot_sb = otp.tile([128, PAIRS, N], bf16)  # O^T rows: pair p
            qt_all = otp.tile([128, 4, N], bf16, name="qt_all")
            kt_all = otp.tile([128, 4, N], bf16, name="kt_all")

            # ---- phase A helpers ----
            def emit_qkt_chains(jobs):
                for xt, mt, ach in jobs:
                    qps = psp.tile([128, 512], f32, tag="acc", bufs=1, name="qps")
                    for ct in range(CT):
                        nc.tensor.matmul(
                            qps, wqk_sb[:, ct, 128 * mt : 128 * (mt + 1)],
                            xt[:, ct, :], start=(ct == 0), stop=(ct == CT - 1))
                    dst = qt_all if mt < 4 else kt_all
                    nc.scalar.copy(
                        dst[:, mt % 4, ACH * ach : ACH * (ach + 1)], qps)

            def load_xt(ach):
                if ach == 0:
                    return xt0
                xt = xtp.tile([128, CT, ACH], bf16, tag="xt", name="xt")
                nc.sync.dma_start(xt, xT_r[:, :, ACH * ach : ACH * (ach + 1)])
                return xt

            def emit_v_group_on(xt, ach, subs):
                for sub in subs:
                    nt = (ACH // 128) * ach + sub
                    vps = psp.tile([128, 512], f32, tag="pvacc", bufs=2,
                                   name="vps")
                    for ct in range(CT):
                        nc.tensor.matmul(vps, xt[:, ct, 128 * sub : 128 * (sub + 1)],
                                         wv_sb[:, ct, :], start=(ct == 0),
                                         stop=(ct == CT - 1))
                    nc.scalar.copy(
                        v_sb[:, nt, :, :],
                        vps.rearrange("p (h d) -> p h d", h=HPC),
                    )

            # ---- phase A pass 1: minimum to start attention ----
            # K^T pair0 (full n), Q^T (pair0, chunk0), V for chunks 0-1.
            # Each x chunk is used and released before the next loads, so
            # iteration-0 S-quads (needing only K0+Q0) can start early.
            xta = load_xt(0)
            emit_qkt_chains([(xta, 4, 0), (xta, 0, 0)])
            emit_v_group_on(xta, 0, range(4))
            xta = load_xt(1)
            emit_qkt_chains([(xta, 4, 1)])
            emit_v_group_on(xta, 1, range(4))
            xta = load_xt(2)
            emit_qkt_chains([(xta, 4, 2)])
            xta = load_xt(3)
            emit_qkt_chains([(xta, 4, 3)])

            def emit_pv_block(st, ks):
                p0, pt, pvps, dens = st
                for k in ks:
                    for hh in range(2):
                        nc.tensor.matmul(
                            pvps[64 * hh : 64 * (hh + 1), :],
                            v_sb[:, k, 2 * p0 + hh, :],
                            pt[:, hh, k, :],
                            start=(k == 0), stop=(k == KT - 1))

            def emit_den_block(st, ks):
                p0, pt, pvps, dens = st
                for k in ks:
                    for hh in range(2):
                        nc.tensor.matmul(
                            dens[32 * hh : 32 * hh + 1, :],
                            ones_sb[:, 0:1],
                            pt[:, hh, k, :],
                            start=(k == 0), stop=(k == KT - 1))

            def emit_norm(st, qc0):
                p0, pt, pvps, dens = st
                rcp = miscp.tile([33, QCH], f32, tag="rcp", bufs=2, name="rcp")
                nc.vector.reciprocal_approx_fast(rcp, dens[0:33, :])
                rcp_d = dp.tile([2, QCH], f32, tag="rcpd", bufs=4, name="rcpd")
                nc.gpsimd.dma_start(rcp_d[0:1, :], rcp[0:1, :])
                nc.gpsimd.dma_start(rcp_d[1:2, :], rcp[32:33, :])
                bc = miscp.tile([128, QCH], f32, tag="bc", bufs=2, name="bc")
                for hh in range(2):
                    rap = rcp_d[hh : hh + 1, :]
                    nc.gpsimd.dma_start(
                        bc[64 * hh : 64 * (hh + 1), :],
                        bass.AP(tensor=rap.tensor, offset=rap.offset,
                                ap=[[0, 64]] + list(rap.ap[1:])),
                    )
                nc.vector.tensor_mul(
                    ot_sb[:, p0, QCH * qc0 : QCH * (qc0 + 1)], pvps, bc)

            def emit_proj_half(qc0, half):
                sub = half
                nt = (QCH // 128) * qc0 + sub * 2
                for nt2 in (nt, nt + 1):
                    for yc in range(2):
                        yps = psp.tile([128, 512], f32, tag="acc", bufs=1,
                                       name="yps")
                        for ot in range(4):
                            nc.tensor.matmul(
                                yps, ot_sb[:, ot, 128 * nt2 : 128 * (nt2 + 1)],
                                wp_sb[:, ot, 512 * yc : 512 * (yc + 1)],
                                start=(ot == 0), stop=(ot == 3))
                        stg = miscp.tile([128, 512], f32, tag="ystg", bufs=2,
                                         name="ystg")
                        nc.scalar.copy(stg, yps)
                        nc.sync.dma_start(
                            y_d[128 * nt2 : 128 * (nt2 + 1), 512 * yc : 512 * (yc + 1)],
                            stg,
                        )

            def qkt_thunk(ach, mts):
                def t():
                    xt = load_xt(ach)
                    emit_qkt_chains([(xt, mt, ach) for mt in mts])
                return t

            def v_thunk(ach, subs):
                def t():
                    xt = load_xt(ach)
                    emit_v_group_on(xt, ach, subs)
                return t

            def wp_thunk():
                def t():
                    for ot in range(4):
                        nc.sync.dma_start(wp_sb[:, ot, :],
                                          wp_d[128 * ot : 128 * (ot + 1), :])
                return t

            def proj_thunk(qc0, half):
                return lambda: emit_proj_half(qc0, half)

            # filler thunks per iteration (iter = 4*qc + p), fired at
            # k = 1, 5, 9, 13. Deps: K^T pair p+1 before iter p+1;
            # Q^T (mt p, chunk qc) before iter 4qc+p; V fully before iter 1
            # (first PV); proj(qc) after norm(qc, pair3) completes.
            EXTRAS = {
                0: [qkt_thunk(0, [5, 1]), qkt_thunk(1, [5]), qkt_thunk(2, [5]),
                    qkt_thunk(3, [5]), v_thunk(2, range(4)),
                    v_thunk(3, range(4))],
                1: [qkt_thunk(0, [6, 2]), qkt_thunk(1, [6]), qkt_thunk(2, [6]),
                    qkt_thunk(3, [6]), wp_thunk()],
                2: [qkt_thunk(0, [7, 3]), qkt_thunk(1, [7]), qkt_thunk(2, [7]),
                    qkt_thunk(3, [7])],
                3: [qkt_thunk(1, [0, 1]), qkt_thunk(1, [2, 3])],
                4: [qkt_thunk(2, [0, 1]), qkt_thunk(2, [2, 3])],
                5: [qkt_thunk(3, [0, 1]), qkt_thunk(3, [2, 3]),
                    proj_thunk(0, 0), proj_thunk(0, 1)],
                9: [proj_thunk(1, 0), proj_thunk(1, 1)],
                13: [proj_thunk(2, 0), proj_thunk(2, 1)],
            }

            # fire PV blocks of the previous iteration at these ks
            PV_AT = {2: range(0, 4), 6: range(4, 8), 10: range(8, 12),
                     14: range(12, 16)}
            DEN_AT = {4: range(0, 8), 12: range(8, 16)}

            pv_st = None
            pv_qc = None
            it = -1
            for qc in range(NQC):
                for p in range(PAIRS):
                    it += 1
                    kt_sb = kt_all[:, p, :]
                    qt_sb = qt_all[:, p, QCH * qc : QCH * (qc + 1)]
                    extras = EXTRAS.get(it, [])
                    ptAB = ptp.tile([128, 2, KT, QCH], bf16, tag="pt", bufs=2,
                                    name="ptab")
                    for k in range(KT):
                        sreg = psp.tile([128, 2, QCH], f32, tag="sreg", bufs=2,
                                        name="sreg")
                        for hh in range(2):
                            sl = slice(64 * hh, 64 * (hh + 1))
                            for sub in range(2):
                                nc.tensor.matmul(
                                    sreg[64 * sub : 64 * (sub + 1), hh, :],
                                    kt_sb[sl, 128 * k + 64 * sub : 128 * k + 64 * (sub + 1)],
                                    qt_sb[sl, :], start=True, stop=True)
                        if k in S_DVE_KS:
                            nc.vector._custom_dve(
                                EXP_POLY,
                                out=ptAB[:, :, k, :],
                                in0=sreg[:],
                                s0=float(SCALE) / 2.0,
                                s1=1.0 / 6.0,
                                imm2=0.5,
                            )
                        else:
                            nc.scalar.activation(
                                out=ptAB[:, :, k, :],
                                in_=sreg[:],
                                func=EXP,
                                scale=float(SCALE),
                            )
                        if pv_st is not None and k in PV_AT:
                            emit_pv_block(pv_st, PV_AT[k])
                        if pv_st is not None and k in DEN_AT:
                            emit_den_block(pv_st, DEN_AT[k])
                        if k == 15 and pv_st is not None:
                            emit_norm(pv_st, pv_qc)
                        if k % 2 == 1 and (k - 1) // 2 < len(extras):
                            extras[(k - 1) // 2]()
                    pvps = psp.tile([128, QCH], f32, tag="pvacc", bufs=2,
                                    name="pvps")
                    dens = psp.tile([33, QCH], f32, tag="dens", bufs=1,
                                    name="dens")
                    pv_st = (p, ptAB, pvps, dens)
                    pv_qc = qc
            # drain the last (qc3, pair3)
            for ks in (range(0, 4), range(4, 8), range(8, 12), range(12, 16)):
                emit_pv_block(pv_st, ks)
            emit_den_block(pv_st, range(0, 8))
            emit_den_block(pv_st, range(8, 16))
            emit_norm(pv_st, pv_qc)
            emit_proj_half(3, 0)
            emit_proj_half(3, 1)

    nc.compile()
    return nc


def get_nc():
    if "nc" not in _CACHE:
        _CACHE["nc"] = _build_nc()
    return _CACHE["nc"]


def make_in_maps(x, w_qkv, w_proj):
    import ml_dtypes

    bf = ml_dtypes.bfloat16
    in_maps = []
    for c in range(8):
        b, g = c // 2, c % 2
        in_maps.append({
            "xT": np.ascontiguousarray(x[b].T).astype(bf),
            "wqk": np.ascontiguousarray(
                np.concatenate(
                    [w_qkv[:, 512 * g : 512 * (g + 1)],
                     w_qkv[:, 1024 + 512 * g : 1024 + 512 * (g + 1)]], axis=1
                )).astype(bf),
            "wv": np.ascontiguousarray(
                w_qkv[:, 2048 + 512 * g : 2048 + 512 * (g + 1)]).astype(bf),
            "wp": np.ascontiguousarray(
                w_proj[512 * g : 512 * (g + 1), :]).astype(bf),
        })
    return in_maps


def kernel(x, w_qkv, w_proj, b_proj):
    from concourse.bass_utils import run_bass_kernel_spmd

    x = np.asarray(x, dtype=np.float32)
    w_qkv = np.asarray(w_qkv, dtype=np.float32)
    w_proj = np.asarray(w_proj, dtype=np.float32)
    b_proj = np.asarray(b_proj, dtype=np.float32)

    nc = get_nc()
    in_maps = make_in_maps(x, w_qkv, w_proj)
    res = run_bass_kernel_spmd(nc, in_maps, list(range(8))).results

    out = np.zeros((B, N, DIM), dtype=np.float32)
    for c in range(8):
        out[c // 2] += res[c]["y"]
    return out + b_proj
